# revision 32
# baseline (speedup 1.0000x reference)
"""BiSeparableConv (ternary depthwise 3x3 + ternary pointwise 1x1) on 8 TRN2 cores.

Math (folded on host):
  m_dw[c]  = max(mean|w_dw[c]|, EPS)            per-channel depthwise scale
  u_dw     = clip(round(w_dw / m_dw), -1, 1)    ternary taps
  M_pw     = max(mean|w_pw|, EPS)               global pointwise scale
  u_pw     = clip(round(w_pw / M_pw), -1, 1)
  y[n,o,s] = sum_c Wt[o,c] * z[n,c,s]           Wt = M_pw * u_pw * m_dw[c]
  z[n,c,s] = sum_t u_dw[c,t] * x[n,c,s+d_t]     9-tap depthwise, pad=1

Device (per core, 2 images, fp16 compute, fp32 PSUM accum):
  - x host-padded to 58x58 rows (zero border), fp16.
  - depthwise split across two lanes:
      DVE : img0 rows [0, DVE_ROWS[b]) = (48, 48, 32) per block, each as
            ONE full span: 9 tensor_scalar muls (4x mode) into a
            contiguous tmp tile, then a pairwise tree of 4 tensor_tensor
            adds (2x mode) -- big FDs amortize the ~310ns per-op
            dispatch+drain.  b2 stops at 32 so the pw chunks gated by
            all three blocks unblock before the PE needs them.
      PE  : img1 (all rows) + img0 tail rows via diagonal-matrix
            matmuls (9 taps accumulated in PSUM), ACT copies PSUM->SBUF.
  - pointwise: 3x3 blocked matmul (K=384) over N=448 spatial chunks,
    PSUM chunk-pairs copied out by ACT; img0 chunks that depend on
    DVE-produced z are scheduled last so the PE never waits.
  - DMA issue spread across Sync/ACT/GpSimd queues (each DMA_DIRECT2D
    issue costs ~0.6-1.1us serially per queue); HAM warmup matmuls fill
    the PE until the first transfers land so the real stream runs warm.
  - y written fp16, host upcasts to fp32.

Rejected by measurement: fp8 x for the depthwise (e4m3 quantization of
x alone gives scale_rel 2.5e-2 > the 2e-2 gate), scalar_tensor_tensor
FMA (1x mode only - slower than mul+add at big FD), matmul N>512
(PSUM dst must fit one 2KB bank), ACT tap-muls (ACT is 1 elem/cyc and
its FIFO couples muls with the PSUM copies), GPSIMD elementwise
(2.6 cyc/elem, shares the DVE SBUF port, no PSUM access).

Host: cached jitted PJRT runner (no per-call re-jit), device-resident
weights, device-created zero donation buffers, threaded pad/cast and
fetch/upcast.
"""

import numpy as np

# ---------------------------------------------------------------- constants
N_CORES = 8
IMGS = 16
IMG_PER_CORE = 2
C = 384
BLK = 3          # channel blocks of 128
H = W = 56
WP = 58          # padded row width / padded row count
PADLEN = WP * WP           # 3364
SLEN = H * W               # 3136
L = PADLEN - (2 * WP + 2)  # 3246: flat-z valid range
EPS = 1e-5

DVE_ROWS = (48, 48, 32)  # img0 rows [0, DVE_ROWS[b]) on DVE per block;
                         # the rest + all of img1 on PE.  b2 stops at 32 so
                         # the pw chunks gated by ALL blocks (rows < 32)
                         # unblock early, while rows 32-48 need only b0/b1.
CHUNK = 8        # pointwise / PE-dw chunk rows (N=448)
XSPLIT = 51      # x DMA band boundary (rows); DVE span [0,48) reads to 50

TAPS = [(dh, dw) for dh in range(3) for dw in range(3)]
DELTA = {t: WP * t[0] + t[1] for t in TAPS}

_cache = {}


def _build(nc_mod, reps=1):
    bass, bacc, tile, mybir = nc_mod
    f16 = mybir.dt.float16
    f32 = mybir.dt.float32
    f8 = mybir.dt.float8e4
    ALU = mybir.AluOpType

    nc = bacc.Bacc(
        "TRN2", target_bir_lowering=False, debug=False, num_devices=N_CORES
    )

    x_d = nc.dram_tensor("x", [IMG_PER_CORE * BLK, 128, PADLEN], f16,
                         kind="ExternalInput")
    wt_d = nc.dram_tensor("wt", [128, BLK * BLK * 128], f16,
                          kind="ExternalInput")
    dg_d = nc.dram_tensor("dg", [128, BLK * 9 * 128], f8,
                          kind="ExternalInput")
    sc_d = nc.dram_tensor("sc", [128, BLK * 9], f32, kind="ExternalInput")
    y_d = nc.dram_tensor("y", [IMG_PER_CORE * BLK, 128, SLEN], f16,
                         kind="ExternalOutput")

    with tile.TileContext(nc) as tc:
        with (
            tc.tile_pool(name="xa", bufs=1) as xa_pool,
            tc.tile_pool(name="zz", bufs=1) as z_pool,
            tc.tile_pool(name="yy", bufs=1) as y_pool,
            tc.tile_pool(name="tmp", bufs=1) as tmp_pool,
            tc.tile_pool(name="wts", bufs=1) as w_pool,
            tc.tile_pool(name="dwps", bufs=2, space="PSUM") as dwps,
            tc.tile_pool(name="pwps", bufs=3, space="PSUM") as pwps,
        ):
            xa = [xa_pool.tile([128, PADLEN], f16, tag=f"xa{u}", name=f"xa{u}")
                  for u in range(6)]
            z = [z_pool.tile([128, PADLEN], f16, tag=f"z{u}", name=f"z{u}")
                 for u in range(6)]
            ym = [y_pool.tile([128, BLK * SLEN], f16, tag=f"ym{i}",
                              name=f"ym{i}") for i in range(2)]
            wt = w_pool.tile([128, BLK * BLK * 128], f16, tag="wt", name="wt")
            dg = w_pool.tile([128, BLK * 9 * 128], f8, tag="dg", name="dg")
            sc = w_pool.tile([128, BLK * 9], f32, tag="sc", name="sc")

            def scal(b, t):
                return sc[:, b * 9 + TAPS.index(t), None]

            def wt_ap(kb, mb):
                i = kb * BLK + mb
                return wt[:, 128 * i:128 * (i + 1)]

            def dg_ap(b, t):
                i = b * 9 + TAPS.index(t)
                return dg[:, 128 * i:128 * (i + 1)]

            def unit(img, b):
                return img * BLK + b

            warm = w_pool.tile([128, 512], f16, tag="warm", name="warm")

            for _rep in range(reps):
                # ---- DMA in: issue spread across engine queues (each
                # DMA_DIRECT2D issue costs ~0.6-1.1us; the Sync queue alone
                # serializes them).  PE-critical (dg b0 + xa3 head) and
                # DVE-critical (xa0 band1) transfers are issued from the
                # engines that come out of the preamble earliest.
                xs = WP * XSPLIT
                DG = 9 * 128
                x12 = WP * 20  # head covers dw chunks r0=0,8 (rows 0..17)
                nc.scalar.dma_start(out=dg[:, :DG], in_=dg_d[:, :DG])
                nc.sync.dma_start(out=xa[3][:, :x12], in_=x_d[3][:, :x12])
                nc.gpsimd.dma_start(out=xa[0][:, :xs], in_=x_d[0][:, :xs])
                nc.sync.dma_start(out=xa[3][:, x12:xs], in_=x_d[3][:, x12:xs])
                nc.sync.dma_start(out=sc[:], in_=sc_d[:])
                nc.sync.dma_start(out=dg[:, DG:2 * DG], in_=dg_d[:, DG:2 * DG])
                nc.gpsimd.dma_start(out=xa[4][:, :xs], in_=x_d[4][:, :xs])
                nc.sync.dma_start(out=dg[:, 2 * DG:], in_=dg_d[:, 2 * DG:])
                nc.gpsimd.dma_start(out=xa[1][:, :xs], in_=x_d[1][:, :xs])
                nc.sync.dma_start(out=xa[5][:, :xs], in_=x_d[5][:, :xs])
                nc.gpsimd.dma_start(out=xa[3][:, xs:], in_=x_d[3][:, xs:])
                nc.sync.dma_start(out=xa[2][:, :xs], in_=x_d[2][:, :xs])
                nc.gpsimd.dma_start(out=wt[:], in_=wt_d[:])
                nc.sync.dma_start(out=xa[0][:, xs:], in_=x_d[0][:, xs:])
                for u, eng in ((4, "g"), (5, "s"), (1, "g"), (2, "s")):
                    e = nc.gpsimd if eng == "g" else nc.sync
                    e.dma_start(out=xa[u][:, xs:], in_=x_d[u][:, xs:])

                # HAM warmup: matmuls with no DMA dependency fill the PE
                # from preamble-end until the first x/dg transfers land
                # (~4.5us), so HAM un-throttles before the real dw stream.
                nc.vector.memset(warm[:], 0)
                wps = dwps.tile([128, 512], f32, tag="dwps", name="dwps")
                for wi in range(9):
                    nc.tensor.matmul(wps[:, :512], warm[:, :128], warm[:],
                                     start=(wi == 0), stop=(wi == 8))

                def ts_tt_chain(u, b, p0, p1):
                    """dw tap sum on DVE: 9 ts-muls (4x mode) + pairwise
                    tree of 4 tt-adds (2x mode) over one contiguous tmp
                    tile; fewer ops = less dispatch+drain overhead."""
                    F = p1 - p0
                    zr = z[u][:, p0:p1]
                    d = DELTA[TAPS[0]]
                    nc.vector.tensor_scalar_mul(
                        zr, xa[u][:, p0 + d:p1 + d], scal(b, TAPS[0]))
                    T = tmp_pool.tile([128, 8 * F], f16, tag="tmp",
                                      name="tmp")
                    for i, t in enumerate(TAPS[1:]):
                        d = DELTA[t]
                        nc.vector.tensor_scalar_mul(
                            T[:, i * F:(i + 1) * F],
                            xa[u][:, p0 + d:p1 + d], scal(b, t))
                    nc.vector.tensor_tensor(
                        T[:, :4 * F], T[:, :4 * F], T[:, 4 * F:8 * F],
                        ALU.add)
                    nc.vector.tensor_tensor(
                        T[:, :2 * F], T[:, :2 * F], T[:, 2 * F:4 * F],
                        ALU.add)
                    nc.vector.tensor_tensor(
                        T[:, :F], T[:, :F], T[:, F:2 * F], ALU.add)
                    nc.vector.tensor_tensor(zr, zr, T[:, :F], ALU.add)


                # ---- DVE: img0 rows [0, DVE_ROWS[b]), one full span per
                # block (amortizes per-op dispatch+drain over the big FD)
                for b in range(BLK):
                    ts_tt_chain(unit(0, b), b, 0, WP * DVE_ROWS[b])

                # ---- PE: img1 dw band1 chunks b-major, then band2
                # chunks (xa band2 arrives later), then img0 tail rows
                band1 = [r0 for r0 in range(0, H, CHUNK)
                         if r0 + CHUNK + 2 <= XSPLIT]
                band2 = [r0 for r0 in range(0, H, CHUNK) if r0 not in band1]
                rblist = [(1, r0, b) for b in range(BLK) for r0 in band1]
                rblist += [(1, r0, b) for b in range(BLK) for r0 in band2]
                rblist += [(0, r0, b) for b in range(BLK)
                           for r0 in range(DVE_ROWS[b], H, CHUNK)
                           if r0 + CHUNK + 2 <= XSPLIT]
                rblist += [(0, r0, b) for b in range(BLK)
                           for r0 in range(DVE_ROWS[b], H, CHUNK)
                           if r0 + CHUNK + 2 > XSPLIT]
                for im, r0, b in rblist:
                    lo = r0
                    nrow = min(CHUNK, H - lo)
                    u = unit(im, b)
                    x3 = xa[u].rearrange("p (h w) -> p h w", w=WP)
                    ps = dwps.tile([128, 512], f32, tag="dwps", name="dwps")
                    dst = ps[:, :nrow * W]
                    for i, t in enumerate(TAPS):
                        dh, dw = t
                        rhs = x3[:, lo + dh:lo + dh + nrow, dw:dw + W]
                        nc.tensor.matmul(dst, dg_ap(b, t), rhs,
                                         start=(i == 0), stop=(i == 8))
                    z3 = z[u].rearrange("p (h w) -> p h w", w=WP)
                    nc.scalar.copy(z3[:, lo:lo + nrow, 0:W], dst)

                # ---- pointwise: chunk pairs into 2-bank PSUM, one ACT copy
                def pw_pair(img, chunks):
                    for mb in range(BLK):
                        ps = pwps.tile([128, 1024], f32, tag="pwps",
                                       name="pwps")
                        for half, r0 in enumerate(chunks):
                            nrow = min(CHUNK, H - r0)
                            dst = ps[:, 512 * half:512 * half + nrow * W]
                            for kb in range(BLK):
                                zk = z[unit(img, kb)].rearrange(
                                    "p (h w) -> p h w", w=WP)
                                rhs = zk[:, r0:r0 + nrow, 0:W]
                                nc.tensor.matmul(dst, wt_ap(kb, mb), rhs,
                                                 start=(kb == 0),
                                                 stop=(kb == 2))
                        r0 = chunks[0]
                        yo = mb * SLEN
                        if len(chunks) == 2:
                            src_ap = ps.rearrange("p (a q) -> p a q", q=512)[
                                :, 0:2, 0:CHUNK * W]
                            dst_ap = ym[img][
                                :, yo + W * r0:yo + W * r0 + 2 * CHUNK * W
                            ].rearrange("p (a q) -> p a q", q=CHUNK * W)
                            nc.scalar.copy(dst_ap, src_ap)
                        else:
                            dst1 = ym[img][:, yo + W * r0:yo + W * (r0 + CHUNK)]
                            nc.scalar.copy(dst1, ps[:, :CHUNK * W])

                for pair in ([0, 8], [16, 24], [32, 40], [48]):
                    pw_pair(1, pair)
                # img0 by gate time: [40,48] and [32] need only b0/b1 DVE
                # spans (+ PE tails); [16,24] / [0,8] wait on the b2 span.
                # [32] goes last so the kernel tail ends on a single-chunk
                # copy + a small 8-row y band.
                for pair in ([40, 48], [16, 24], [0, 8], [32]):
                    pw_pair(0, pair)

                # ---- DMA out: one DMA per row-region covering all 3 mb;
                # issue from different queues so a band's sem-wait doesn't
                # stall the later bands' issues.
                def y_out(img, c0, c1, eng):
                    a, b2_ = W * c0, W * c1
                    dst = y_d[img * BLK:(img + 1) * BLK, :, a:b2_].rearrange(
                        "i p q -> p i q")
                    src_ = ym[img].rearrange("p (i q) -> p i q", q=SLEN)[
                        :, :, a:b2_]
                    eng.dma_start(out=dst, in_=src_)

                for c0, c1 in ((0, 28), (28, 56)):
                    y_out(1, c0, c1, nc.sync)
                y_out(0, 40, 56, nc.sync)
                # late img0 bands split per-mb: each slice leaves as soon
                # as its own pw copy lands instead of waiting for all
                # three.  No issues on ACT -- they'd sit behind its
                # remaining copies in the FIFO and fire late.
                for c0, c1 in ((16, 32), (0, 16), (32, 40)):
                    a, b2_ = W * c0, W * c1
                    for mb in range(BLK):
                        eng = nc.gpsimd if mb == 0 else nc.sync
                        eng.dma_start(
                            out=y_d[mb, :, a:b2_],
                            in_=ym[0][:, mb * SLEN + a:mb * SLEN + b2_])

    nc.compile()
    return nc


def _get_nc(reps=1):
    key = ("nc", reps)
    if key not in _cache:
        import concourse.bass as bass
        import concourse.bacc as bacc
        import concourse.tile as tile
        import concourse.mybir as mybir
        _cache[key] = _build((bass, bacc, tile, mybir), reps)
        if reps == 1:
            _cache["nc"] = _cache[key]
    return _cache[key]


def _pool():
    if "pool" not in _cache:
        from concurrent.futures import ThreadPoolExecutor
        _cache["pool"] = ThreadPoolExecutor(max_workers=8)
    return _cache["pool"]


def _get_runner():
    """Build the jitted shard_map callable ONCE; re-jitting per call costs
    seconds (trace + XLA + executable load over the PJRT tunnel)."""
    if "runner" in _cache:
        return _cache["runner"]
    nc = _get_nc()
    import jax
    import jax.numpy as jnp
    from jax.sharding import Mesh, PartitionSpec, NamedSharding
    from jax.experimental.shard_map import shard_map
    from concourse import mybir
    from concourse.bass2jax import (_bass_exec_p, install_neuronx_cc_hook,
                                    partition_id_tensor)

    install_neuronx_cc_hook()
    partition_name = (nc.partition_id_tensor.name
                      if nc.partition_id_tensor else None)
    in_names, out_names, out_avals, zero_shapes = [], [], [], []
    for alloc in nc.m.functions[0].allocations:
        if not isinstance(alloc, mybir.MemoryLocationSet):
            continue
        name = alloc.memorylocations[0].name
        if alloc.kind == "ExternalInput":
            if name != partition_name:
                in_names.append(name)
        elif alloc.kind == "ExternalOutput":
            out_names.append(name)
            shape = tuple(alloc.tensor_shape)
            dtype = mybir.dt.np(alloc.dtype)
            out_avals.append(jax.core.ShapedArray(shape, dtype))
            zero_shapes.append((shape, dtype))
    n_params = len(in_names)
    all_names = list(in_names) + list(out_names)
    if partition_name is not None:
        all_names.append(partition_name)

    mesh = Mesh(np.asarray(jax.devices()[:N_CORES]), ("core",))
    sh = NamedSharding(mesh, PartitionSpec("core"))

    def _body(*args):
        operands = list(args)
        if partition_name is not None:
            operands.append(partition_id_tensor())
        return tuple(_bass_exec_p.bind(
            *operands,
            out_avals=tuple(out_avals),
            in_names=tuple(all_names),
            out_names=tuple(out_names),
            lowering_input_output_aliases=(),
            sim_require_finite=True,
            sim_require_nnan=True,
            nc=nc,
        ))

    n_outs = len(out_names)
    sharded = jax.jit(
        shard_map(_body, mesh=mesh,
                  in_specs=(PartitionSpec("core"),) * (n_params + n_outs),
                  out_specs=(PartitionSpec("core"),) * n_outs,
                  check_rep=False),
        donate_argnums=tuple(range(n_params, n_params + n_outs)),
        keep_unused=True)

    mkzeros = jax.jit(lambda: tuple(
        jax.lax.with_sharding_constraint(
            jnp.zeros((N_CORES * s[0],) + tuple(s[1:]), d), sh)
        for s, d in zero_shapes))

    def run(arrays, device_arrays):
        args = [device_arrays[nm] if nm in device_arrays
                else jax.device_put(arrays[nm], sh) for nm in in_names]
        zeros = _cache.pop("next_zeros", None)
        if zeros is None:
            zeros = mkzeros()
        outs = sharded(*args, *zeros)
        _cache["next_zeros"] = mkzeros()  # async; ready before next call
        return outs

    def put(arr):
        return jax.device_put(arr, sh)

    _cache["runner"] = (run, put)
    return _cache["runner"]


def _prep_weights(w_dw, w_pw):
    """Fold quantization scales into the matmul weights; cache the device
    copies keyed by the raw weight bytes (w tensors are tiny)."""
    import hashlib
    key = hashlib.blake2b(w_dw.tobytes() + w_pw.tobytes(),
                          digest_size=16).digest()
    ent = _cache.get("weights")
    if ent is not None and ent[0] == key:
        return ent[1], ent[2]

    m = np.maximum(np.mean(np.abs(w_dw.reshape(C, -1)), axis=1,
                           dtype=np.float32), EPS)            # [C]
    u_dw = np.clip(np.round(w_dw[:, 0] * (1.0 / m)[:, None, None]), -1, 1)
    M_pw = max(np.mean(np.abs(w_pw), dtype=np.float32), np.float32(EPS))
    u_pw = np.clip(np.round(w_pw[:, :, 0, 0] * (1.0 / M_pw)), -1, 1)
    Wt = (u_pw * (m * np.float32(M_pw))[None, :]).astype(np.float16)  # [O,C]

    # lhsT layout: wt16[k_part, (kb mb m)] = Wt[mb*128+m, kb*128+k_part]
    wt4 = Wt.reshape(BLK, 128, BLK, 128)            # [mb, mo, kb, ki]
    wt16 = np.ascontiguousarray(
        wt4.transpose(3, 2, 0, 1).reshape(128, BLK * BLK * 128))

    # diag tiles: dg8[p, (b t m)] = u_dw[b*128+m, t] if p == m else 0.
    # Ternary taps are exact in fp8 e4m3; fp8 lhsT with fp16 rhs is a
    # legal mixed-dtype matmul and halves the head-critical dg DMA.
    from ml_dtypes import float8_e4m3fn
    u8 = u_dw.astype(float8_e4m3fn).reshape(BLK, 128, 9)   # [b, c, t]
    dg4 = np.zeros((128, BLK, 9, 128), dtype=float8_e4m3fn)
    idx = np.arange(128)
    dg4[idx, :, :, idx] = u8.transpose(1, 0, 2)
    dg16 = np.ascontiguousarray(dg4.reshape(128, BLK * 9 * 128))

    sc32 = np.ascontiguousarray(
        u_dw.astype(np.float32).reshape(BLK, 128, 9).transpose(1, 0, 2)
        .reshape(128, BLK * 9))

    host = {"wt": wt16, "dg": dg16, "sc": sc32}
    _cache["weights"] = (key, host, {})
    return host, {}


def _dev_weights(host_w):
    """Device-resident per-core-tiled weight copies (fast path only)."""
    key, host, dev = _cache["weights"]
    if not dev:
        run, put = _get_runner()
        dev = {nm: put(np.ascontiguousarray(
            np.broadcast_to(a, (N_CORES,) + a.shape)
            .reshape(N_CORES * a.shape[0], a.shape[1])))
            for nm, a in host.items()}
        _cache["weights"] = (key, host, dev)
    return dev


def _prep_x(x):
    """fp32 [16,C,56,56] -> padded fp16 [48,128,3364], threaded cast;
    the zero border is written once and reused across calls."""
    xp = _cache.get("xpad")
    if xp is None:
        xp = np.zeros((IMGS, BLK, 128, WP, WP), dtype=np.float16)
        _cache["xpad"] = xp
    dst = xp[:, :, :, 1:H + 1, 1:W + 1]
    src = x.reshape(IMGS, BLK, 128, H, W)

    def cp(i):
        np.copyto(dst[i], src[i], casting="same_kind")

    list(_pool().map(cp, range(IMGS)))
    return xp.reshape(IMGS * BLK, 128, PADLEN)


def kernel(x: np.ndarray, w_dw: np.ndarray, w_pw: np.ndarray) -> np.ndarray:
    assert x.shape == (IMGS, C, H, W) and x.dtype == np.float32
    host_w, dev_w = _prep_weights(w_dw, w_pw)
    xg = _prep_x(x)                       # [48, 128, PADLEN] fp16 global

    run_kwargs = _cache.get("run_kwargs", {})
    out = np.empty((IMGS, C, H, W), dtype=np.float32)

    if run_kwargs:
        # profiling path (test.py): run through run_bass_kernel_spmd so
        # NTFF exec time is captured; slower wall-clock (fresh jit).
        nc = _get_nc()
        in_maps = []
        for k in range(N_CORES):
            xin = np.ascontiguousarray(
                xg[IMG_PER_CORE * BLK * k:IMG_PER_CORE * BLK * (k + 1)])
            in_maps.append({"x": xin, **host_w})
        _cache["last_in_maps"] = in_maps
        from concourse import bass_utils
        res = bass_utils.run_bass_kernel_spmd(
            nc, in_maps, list(range(N_CORES)), **run_kwargs)
        _cache["last_results"] = res
        for k in range(N_CORES):
            yk = res.results[k]["y"].reshape(IMG_PER_CORE, BLK * 128, H, W)
            np.copyto(out[IMG_PER_CORE * k:IMG_PER_CORE * (k + 1)], yk,
                      casting="same_kind")
        return out

    # fast path: cached jit, device-resident weights, device-made zero
    # donation buffers, bulk fetch + threaded upcast.
    run, put = _get_runner()
    outs = run({"x": xg}, _dev_weights(host_w))
    y = np.asarray(outs[0]).reshape(IMGS, BLK * 128, H, W)

    def up(i):
        np.copyto(out[i], y[i], casting="same_kind")

    list(_pool().map(up, range(IMGS)))
    return out

